# revision 30
# baseline (speedup 1.0000x reference)
"""Trainium2 Bass kernel for MultiHeadEdgeAwareMessagePassing.

Math restructure (validated vs reference, ~1e-3 final rel err incl. bf16):
  logits[i,j,h] = s_q[i,h] + s_k[j,h] + w[i,j]*c1[h] + c0[h]   (valid j: w>0)
  alpha = softmax_j(logits) * w
s_q, c0 are constant over j and cancel in the softmax; bk's contribution to
s_k scales numerator and denominator equally and cancels too. With
g[j,h] = exp(h[j]@a_k[h]), a_k[h] = u_k[h] @ Wk[h-block], v = h@Wv^T + bv:
  msg[i,h,:] = Num_h[i,:] / Den_h[i]
  Num_h = W1^T (g_h*v_h)
  Den_h = mask^T g_h + c1_h (W1^T g_h)
where mask=[w>0], W1=relu(w)  (exp(c1 w) ~= 1 + c1 w, |c1 w| < 0.02; the
dropped quadratic term changes the final output by ~3e-6 relative).

Sharding: destination rows i split across 8 cores (384 rows each). Each core
reads its [3072, 384] slice of w^T plus replicated h^T and the small weights.
Host-side transposes are layout prep only; all compute runs on device.
"""

import numpy as np

N = 3072
D = 256
H = 4
DH = 64
DE = 8
NCORES = 8
ISLICE = N // NCORES  # 384
NSUB = ISLICE // 128  # 3
CJT = 4               # j-tiles per chunk
NCH = N // (128 * CJT)  # 6 chunks

_cache = {}


def _build_bass():
    import concourse.bass as bass
    import concourse.tile as tile
    from concourse import bacc, mybir
    from concourse.bass import ts
    from concourse.masks import make_identity

    dt = mybir.dt
    AF = mybir.ActivationFunctionType
    OP = mybir.AluOpType

    nc = bacc.Bacc("TRN2", target_bir_lowering=False, debug=False,
                   num_devices=NCORES)

    wt_d = nc.dram_tensor("wt", [N, ISLICE], dt.float32, kind="ExternalInput")
    ht_d = nc.dram_tensor("ht", [D, N], dt.bfloat16, kind="ExternalInput")
    hs_d = nc.dram_tensor("hs", [ISLICE, D], dt.float32, kind="ExternalInput")
    # su1: critical setup consts (bf16): WvT 512 | Wk2 1024 | u4 4 | ue4 4
    #      | wew 4 | bv row0 256  -> 1804 cols
    su1_d = nc.dram_tensor("su1", [128, 1804], dt.bfloat16,
                           kind="ExternalInput")
    # su2a: epilogue bf16 consts: WoT 512 | ident 128 | bo row0 256
    su2a_d = nc.dram_tensor("su2a", [128, 896], dt.bfloat16,
                            kind="ExternalInput")
    # su2b: epilogue f32 consts: gamma 256 | beta 256 (pre-broadcast)
    su2b_d = nc.dram_tensor("su2b", [128, 512], dt.float32,
                            kind="ExternalInput")
    out_d = nc.dram_tensor("out", [ISLICE, D], dt.float32, kind="ExternalOutput")

    bf = dt.bfloat16
    f32 = dt.float32

    with tile.TileContext(nc) as tc:
        with (
            tc.tile_pool(name="consts", bufs=1) as consts,
            tc.tile_pool(name="wtp", bufs=3) as wtp,
            tc.tile_pool(name="elem", bufs=3) as elem,
            tc.tile_pool(name="rhsp", bufs=4) as rhsp,
            tc.tile_pool(name="gp", bufs=3) as gp,
            tc.tile_pool(name="small", bufs=4) as small,
            tc.tile_pool(name="outp", bufs=2) as outp,
            tc.tile_pool(name="acc", bufs=1, space="PSUM") as accp,
            tc.tile_pool(name="pre4", bufs=2, space="PSUM") as pre4,
            tc.tile_pool(name="presk", bufs=1, space="PSUM") as presk,
        ):
            # ---- setup consts: host-packed bf16, one sync DMA, no casts ----
            sbf = consts.tile([128, 1804], bf, tag="sbf")
            nc.sync.dma_start(sbf, su1_d.ap())
            bv_row = sbf[0:1, 1548:1804]
            rhs_wv = sbf[:, 0:512].rearrange("p (a n) -> p a n", a=2)

            ones_sb = consts.tile([1, 128], bf, tag="ones")
            nc.vector.memset(ones_sb, 1.0)
            eps_sb = consts.tile([128, 1], f32, tag="eps")
            nc.vector.memset(eps_sb, 1e-5)

            # ------------- setup matmuls -------------
            # a_k^T[dm, h] = sum_d Wk[h*64+d, dm] u_k[h, d]
            rhs_ak = consts.tile([128, 2, H], bf, tag="rhsak")
            for b in range(2):
                ps_ak = presk.tile([128, H], f32, tag="sk4")
                for h in range(H):
                    nc.tensor.matmul(
                        ps_ak[:, h:h + 1],
                        sbf[0:DH, 512 + h * 256 + 128 * b:
                            512 + h * 256 + 128 * (b + 1)],
                        sbf[0:DH, 1536 + h:1537 + h],
                        start=True, stop=True, skip_group_check=True)
                nc.vector.tensor_copy(rhs_ak[:, b, :], ps_ak)

            # c1[h] = sum_d We_w[h*8+d] u_e[h, d], broadcast to partitions
            ps_c1 = presk.tile([1, H], f32, tag="sk4")
            for h in range(H):
                nc.tensor.matmul(ps_c1[:, h:h + 1],
                                 sbf[0:DE, 1544 + h:1545 + h],
                                 sbf[0:DE, 1540 + h:1541 + h],
                                 start=True, stop=True,
                                 skip_group_check=True)
            c1row = consts.tile([1, H], bf, tag="c1row")
            nc.vector.tensor_copy(c1row, ps_c1)
            ps_c1b = presk.tile([128, H], f32, tag="sk4")
            nc.tensor.matmul(ps_c1b, ones_sb, c1row, start=True, stop=True)
            c1b = consts.tile([128, H], f32, tag="c1b")
            nc.vector.tensor_copy(c1b, ps_c1b)

            # ---------------- persistent accumulators ----------------
            # cols 0:256 = W1.gV, 256:260 = W1.g, 260:264 = mask.g
            psA = [accp.tile([128, 264], f32, tag=f"A{s}", name=f"psA{s}")
                   for s in range(NSUB)]

            ht_sb = consts.tile([128, 2, N], bf, tag="ht")
            ht_re = ht_d.ap().rearrange("(a p) n -> p a n", p=128)

            # DMA order: ht0, wt0, ht1..5 (small, unblocks all preproc),
            # then wt1..5
            nc.sync.dma_start(ht_sb[:, :, ts(0, 128 * CJT)],
                              ht_re[:, :, ts(0, 128 * CJT)])
            wt_tiles = []
            for ch in range(NCH):
                wt_tiles.append(wtp.tile([128, CJT, ISLICE], f32, tag="wt",
                                         name=f"wt4_{ch}"))
            nc.sync.dma_start(
                wt_tiles[0], wt_d[ts(0, 128 * CJT), :].rearrange(
                    "(j p) i -> p j i", p=128))
            for ch in range(1, NCH):
                nc.sync.dma_start(ht_sb[:, :, ts(ch, 128 * CJT)],
                                  ht_re[:, :, ts(ch, 128 * CJT)])
            for ch in range(1, NCH):
                nc.sync.dma_start(
                    wt_tiles[ch], wt_d[ts(ch, 128 * CJT), :].rearrange(
                        "(j p) i -> p j i", p=128))
            hseg_all = consts.tile([128, NSUB, D], f32, tag="hsegall")
            nc.sync.dma_start(
                hseg_all, hs_d.ap().rearrange("(s p) n -> p s n", p=128))

            # ---------------- main loop ----------------
            for ch in range(NCH):
                wt4 = wt_tiles[ch]

                W1c = elem.tile([128, CJT, ISLICE], bf, tag="W1")
                nc.scalar.activation(W1c, wt4, AF.Relu)
                mskc = elem.tile([128, CJT, ISLICE], bf, tag="msk")
                nc.vector.tensor_scalar(mskc, W1c, 0.0, None, op0=OP.is_gt)

                # --- v and s_k for the CJT j-tiles of this chunk ---
                ps_v4 = pre4.tile([128, CJT, 256], f32, tag="v4")
                ps_sk4 = presk.tile([128, CJT, H], f32, tag="sk4")
                for jm in range(CJT):
                    jt = ch * CJT + jm
                    for a in range(2):
                        nc.tensor.matmul(ps_v4[:, jm, :],
                                         ht_sb[:, a, ts(jt, 128)],
                                         rhs_wv[:, a, :],
                                         start=(a == 0), stop=False)
                        nc.tensor.matmul(ps_sk4[:, jm, :],
                                         ht_sb[:, a, ts(jt, 128)],
                                         rhs_ak[:, a, :],
                                         start=(a == 0), stop=(a == 1))
                    nc.tensor.matmul(ps_v4[:, jm, :], ones_sb, bv_row,
                                     start=False, stop=True)

                g32 = gp.tile([128, CJT, H], f32, tag="g32")
                nc.scalar.activation(g32, ps_sk4, AF.Exp)

                rhs4 = rhsp.tile([128, CJT, 260], bf, tag="rhsbig")
                g32b = bass.AP(tensor=g32.tensor, offset=g32.offset,
                               ap=[g32.ap[0], g32.ap[1], g32.ap[2], [0, DH]])
                nc.vector.tensor_tensor(
                    out=rhs4[:, :, 0:256].rearrange(
                        "p j (h d) -> p j h d", h=H),
                    in0=ps_v4.rearrange("p j (h d) -> p j h d", h=H),
                    in1=g32b, op=OP.mult)
                nc.vector.tensor_copy(rhs4[:, :, 256:260], g32)

                st = (ch == 0)
                sp = (ch == NCH - 1)
                for jm in range(CJT):
                    for s in range(NSUB):
                        sl = ts(s, 128)
                        nc.tensor.matmul(psA[s][:, 0:260], W1c[:, jm, sl],
                                         rhs4[:, jm, :], start=st, stop=sp,
                                         skip_group_check=True)
                        nc.tensor.matmul(psA[s][:, 260:264], mskc[:, jm, sl],
                                         rhs4[:, jm, 256:260], start=st, stop=sp,
                                         skip_group_check=True)

            # ---------------- epilogue consts (end of sync queue) -------
            su2a = consts.tile([128, 896], bf, tag="su2a")
            nc.sync.dma_start(su2a, su2a_d.ap())
            su2b = consts.tile([128, 512], f32, tag="su2b")
            nc.sync.dma_start(su2b, su2b_d.ap())
            WoT_sb = su2a[:, 0:512].rearrange("p (a n) -> p a n", a=2)
            ident = su2a[:, 512:640]
            bo_row = su2a[0:1, 640:896]
            gam_sb = su2b[:, 0:256]
            bet_sb = su2b[:, 256:512]

            # ---------------- epilogue ----------------
            rdens = []
            for s in range(NSUB):
                dg = small.tile([128, H], f32, tag="dg", name=f"dg{s}")
                nc.vector.tensor_copy(dg, psA[s][:, 256:260])
                den = small.tile([128, H], f32, tag="den", name=f"den{s}")
                nc.vector.tensor_mul(den, c1b, dg)
                nc.vector.tensor_add(den, den, psA[s][:, 260:264])
                rden = small.tile([128, H], f32, tag="rden", name=f"rden{s}")
                nc.vector.reciprocal(rden, den)
                rdens.append(rden)

            msgs = []
            for s in range(NSUB):
                msg = outp.tile([128, D], bf, tag="msg", name=f"msg{s}")
                for h in range(H):
                    hsl = slice(h * DH, (h + 1) * DH)
                    nc.vector.tensor_scalar(msg[:, hsl], psA[s][:, hsl],
                                            rdens[s][:, h:h + 1], None,
                                            op0=OP.mult)
                msgs.append(msg)

            msgTs = []
            for s in range(NSUB):
                ps_t = pre4.tile([128, 2, 128], bf, tag="v4", name=f"pst{s}")
                for b in range(2):
                    nc.tensor.transpose(ps_t[:, b, :], msgs[s][:, ts(b, 128)],
                                        ident, )
                msgT = outp.tile([128, 2, 128], bf, tag="msgT", name=f"msgT{s}")
                nc.vector.tensor_copy(msgT, ps_t)
                msgTs.append(msgT)

            for s in range(NSUB):
                ps_o = pre4.tile([128, D], f32, tag="v4", name=f"pso{s}")
                nc.tensor.matmul(ps_o, msgTs[s][:, 0, :], WoT_sb[:, 0, :],
                                 start=True, stop=False)
                nc.tensor.matmul(ps_o, msgTs[s][:, 1, :], WoT_sb[:, 1, :],
                                 start=False, stop=False)
                nc.tensor.matmul(ps_o, ones_sb, bo_row, start=False, stop=True)

                x = outp.tile([128, D], f32, tag="x", name=f"x{s}")
                nc.vector.tensor_add(x, ps_o, hseg_all[:, s, :])

                stats = small.tile([128, 6], f32, tag="stats", name=f"st{s}")
                nc.vector.bn_stats(out=stats, in_=x)
                mv = small.tile([128, 2], f32, tag="mv", name=f"mv{s}")
                nc.vector.bn_aggr(out=mv, in_=stats)
                sd = small.tile([128, 1], f32, tag="sd", name=f"sd{s}")
                nc.scalar.activation(sd, mv[:, 1:2], AF.Sqrt, bias=eps_sb)
                rstd = small.tile([128, 1], f32, tag="rstd", name=f"rst{s}")
                nc.vector.reciprocal(rstd, sd)

                y = outp.tile([128, D], f32, tag="y", name=f"y{s}")
                nc.vector.tensor_scalar(y, x, mv[:, 0:1], rstd,
                                        op0=OP.subtract, op1=OP.mult)
                ot = outp.tile([128, D], f32, tag="ot", name=f"ot{s}")
                nc.vector.tensor_mul(ot, y, gam_sb)
                nc.vector.tensor_add(ot, ot, bet_sb)
                nc.sync.dma_start(out_d[ts(s, 128), :], ot)

    nc.compile()
    return nc


def _make_in_maps(h, w, Wk, Wv, bv, We_w, u, Wo, bo, gamma, beta, **_unused):
    import ml_dtypes
    f = np.float32
    b16 = ml_dtypes.bfloat16
    h = np.ascontiguousarray(h, dtype=f)
    wT = np.ascontiguousarray(np.asarray(w, dtype=f).T)
    Wk = np.asarray(Wk, dtype=f)
    u = np.asarray(u, dtype=f)
    We_w = np.asarray(We_w, dtype=f)

    # su1 (bf16): WvT 0:512 | Wk 512:1536 | u_k 1536:1540 | u_e 1540:1544
    #             | We_w 1544:1548 | bv row0 1548:1804
    su1 = np.zeros((128, 1804), f)
    WvT = np.asarray(Wv, dtype=f).T
    su1[:, 0:512] = WvT.reshape(2, 128, D).transpose(1, 0, 2).reshape(128, 512)
    for hh in range(H):
        su1[0:DH, 512 + hh * 256:512 + (hh + 1) * 256] = \
            Wk[hh * DH:(hh + 1) * DH, :]
        su1[0:DH, 1536 + hh] = u[hh, DH:2 * DH]
        su1[0:DE, 1540 + hh] = u[hh, 2 * DH:2 * DH + DE]
        su1[0:DE, 1544 + hh] = We_w[hh * DE:(hh + 1) * DE, 0]
    su1[0, 1548:1804] = np.asarray(bv, dtype=f)

    # su2a (bf16): WoT 0:512 | identity 512:640 | bo row0 640:896
    su2a = np.zeros((128, 896), f)
    WoT = np.asarray(Wo, dtype=f).T
    su2a[:, 0:512] = WoT.reshape(2, 128, D).transpose(1, 0, 2).reshape(128, 512)
    su2a[:, 512:640] = np.eye(128, dtype=f)
    su2a[0, 640:896] = np.asarray(bo, dtype=f)

    # su2b (f32): gamma/beta broadcast to 128 partitions
    su2b = np.zeros((128, 512), f)
    su2b[:, 0:256] = np.asarray(gamma, dtype=f)[None, :]
    su2b[:, 256:512] = np.asarray(beta, dtype=f)[None, :]

    common = {
        "ht": np.ascontiguousarray(h.T.astype(b16)),
        "su1": su1.astype(b16),
        "su2a": su2a.astype(b16),
        "su2b": su2b,
    }
    in_maps = []
    for c in range(NCORES):
        sl = slice(c * ISLICE, (c + 1) * ISLICE)
        m = dict(common)
        m["wt"] = np.ascontiguousarray(wT[:, sl])
        m["hs"] = np.ascontiguousarray(h[sl, :])
        in_maps.append(m)
    return in_maps


def kernel(**inputs):
    from concourse.bass_utils import run_bass_kernel_spmd

    if "nc" not in _cache:
        _cache["nc"] = _build_bass()
    nc = _cache["nc"]

    in_maps = _make_in_maps(**inputs)
    res = run_bass_kernel_spmd(nc, in_maps, core_ids=list(range(NCORES)))
    out = np.concatenate([r["out"] for r in res.results], axis=0)
    return np.ascontiguousarray(out, dtype=np.float32)


# revision 31
# speedup vs baseline: 1.1220x; 1.1220x over previous
"""Trainium2 Bass kernel for MultiHeadEdgeAwareMessagePassing.

Math restructure (validated vs reference, ~1e-3 final rel err incl. bf16):
  logits[i,j,h] = s_q[i,h] + s_k[j,h] + w[i,j]*c1[h] + c0[h]   (valid j: w>0)
  alpha = softmax_j(logits) * w
s_q, c0 are constant over j and cancel in the softmax; bk's contribution to
s_k scales numerator and denominator equally and cancels too. With
g[j,h] = exp(h[j]@a_k[h]), a_k[h] = u_k[h] @ Wk[h-block], v = h@Wv^T + bv:
  msg[i,h,:] = Num_h[i,:] / Den_h[i]
  Num_h = W1^T (g_h*v_h)
  Den_h = mask^T g_h + c1_h (W1^T g_h)
where mask=[w>0], W1=relu(w)  (exp(c1 w) ~= 1 + c1 w, |c1 w| < 0.02; the
dropped quadratic term changes the final output by ~3e-6 relative).

Sharding: destination rows i split across 8 cores (384 rows each). Each core
reads its [3072, 384] slice of w^T plus replicated h^T and the small weights.
Host-side transposes are layout prep only; all compute runs on device.
"""

import numpy as np

N = 3072
D = 256
H = 4
DH = 64
DE = 8
NCORES = 8
ISLICE = N // NCORES  # 384
NSUB = ISLICE // 128  # 3
CJT = 4               # j-tiles per chunk
NCH = N // (128 * CJT)  # 6 chunks

_cache = {}


def _build_bass():
    import concourse.bass as bass
    import concourse.tile as tile
    from concourse import bacc, mybir
    from concourse.bass import ts
    from concourse.masks import make_identity

    dt = mybir.dt
    AF = mybir.ActivationFunctionType
    OP = mybir.AluOpType

    nc = bacc.Bacc("TRN2", target_bir_lowering=False, debug=False,
                   num_devices=NCORES)

    wt_d = nc.dram_tensor("wt", [N, ISLICE], dt.float32, kind="ExternalInput")
    ht_d = nc.dram_tensor("ht", [D, N], dt.bfloat16, kind="ExternalInput")
    hs_d = nc.dram_tensor("hs", [ISLICE, D], dt.float32, kind="ExternalInput")
    # su1: critical setup consts (bf16): WvT 512 | Wk2 1024 | u4 4 | ue4 4
    #      | wew 4 | bv row0 256  -> 1804 cols
    su1_d = nc.dram_tensor("su1", [128, 1804], dt.bfloat16,
                           kind="ExternalInput")
    # su2a: epilogue bf16 consts: WoT 512 | ident 128 | bo row0 256
    su2a_d = nc.dram_tensor("su2a", [128, 896], dt.bfloat16,
                            kind="ExternalInput")
    # su2b: epilogue f32 consts: gamma 256 | beta 256 (pre-broadcast)
    su2b_d = nc.dram_tensor("su2b", [128, 512], dt.float32,
                            kind="ExternalInput")
    out_d = nc.dram_tensor("out", [ISLICE, D], dt.float32, kind="ExternalOutput")

    bf = dt.bfloat16
    f32 = dt.float32

    with tile.TileContext(nc) as tc:
        with (
            tc.tile_pool(name="consts", bufs=1) as consts,
            tc.tile_pool(name="wtp", bufs=4) as wtp,
            tc.tile_pool(name="elem", bufs=4) as elem,
            tc.tile_pool(name="rhsp", bufs=6) as rhsp,
            tc.tile_pool(name="gp", bufs=4) as gp,
            tc.tile_pool(name="small", bufs=4) as small,
            tc.tile_pool(name="outp", bufs=2) as outp,
            tc.tile_pool(name="acc", bufs=1, space="PSUM") as accp,
            tc.tile_pool(name="pre4", bufs=2, space="PSUM") as pre4,
            tc.tile_pool(name="presk", bufs=1, space="PSUM") as presk,
        ):
            # ---- setup consts: host-packed bf16, one sync DMA, no casts ----
            sbf = consts.tile([128, 1804], bf, tag="sbf")
            nc.sync.dma_start(sbf, su1_d.ap())
            bv_row = sbf[0:1, 1548:1804]
            rhs_wv = sbf[:, 0:512].rearrange("p (a n) -> p a n", a=2)

            ones_sb = consts.tile([1, 128], bf, tag="ones")
            nc.vector.memset(ones_sb, 1.0)
            eps_sb = consts.tile([128, 1], f32, tag="eps")
            nc.vector.memset(eps_sb, 1e-5)

            # ------------- setup matmuls -------------
            # a_k^T[dm, h] = sum_d Wk[h*64+d, dm] u_k[h, d]
            rhs_ak = consts.tile([128, 2, H], bf, tag="rhsak")
            for b in range(2):
                ps_ak = presk.tile([128, H], f32, tag="sk4")
                for h in range(H):
                    nc.tensor.matmul(
                        ps_ak[:, h:h + 1],
                        sbf[0:DH, 512 + h * 256 + 128 * b:
                            512 + h * 256 + 128 * (b + 1)],
                        sbf[0:DH, 1536 + h:1537 + h],
                        start=True, stop=True, skip_group_check=True)
                nc.vector.tensor_copy(rhs_ak[:, b, :], ps_ak)

            # c1[h] = sum_d We_w[h*8+d] u_e[h, d], broadcast to partitions
            ps_c1 = presk.tile([1, H], f32, tag="sk4")
            for h in range(H):
                nc.tensor.matmul(ps_c1[:, h:h + 1],
                                 sbf[0:DE, 1544 + h:1545 + h],
                                 sbf[0:DE, 1540 + h:1541 + h],
                                 start=True, stop=True,
                                 skip_group_check=True)
            c1row = consts.tile([1, H], bf, tag="c1row")
            nc.vector.tensor_copy(c1row, ps_c1)
            ps_c1b = presk.tile([128, H], f32, tag="sk4")
            nc.tensor.matmul(ps_c1b, ones_sb, c1row, start=True, stop=True)
            c1b = consts.tile([128, H], f32, tag="c1b")
            nc.vector.tensor_copy(c1b, ps_c1b)

            # ---------------- persistent accumulators ----------------
            # cols 0:256 = W1.gV, 256:260 = W1.g, 260:264 = mask.g
            psA = [accp.tile([128, 264], f32, tag=f"A{s}", name=f"psA{s}")
                   for s in range(NSUB)]

            ht_sb = consts.tile([128, 2, N], bf, tag="ht")
            ht_re = ht_d.ap().rearrange("(a p) n -> p a n", p=128)

            # DMA order: ht0, wt0, ht1..5 (small, unblocks all preproc),
            # then wt1..5
            nc.sync.dma_start(ht_sb[:, :, ts(0, 128 * CJT)],
                              ht_re[:, :, ts(0, 128 * CJT)])
            wt_tiles = []
            for ch in range(NCH):
                wt_tiles.append(wtp.tile([128, CJT, ISLICE], f32, tag="wt",
                                         name=f"wt4_{ch}"))
            nc.sync.dma_start(
                wt_tiles[0], wt_d[ts(0, 128 * CJT), :].rearrange(
                    "(j p) i -> p j i", p=128))
            for ch in range(1, NCH):
                nc.sync.dma_start(ht_sb[:, :, ts(ch, 128 * CJT)],
                                  ht_re[:, :, ts(ch, 128 * CJT)])
            for ch in range(1, NCH):
                nc.sync.dma_start(
                    wt_tiles[ch], wt_d[ts(ch, 128 * CJT), :].rearrange(
                        "(j p) i -> p j i", p=128))
            hseg_all = consts.tile([128, NSUB, D], f32, tag="hsegall")
            nc.sync.dma_start(
                hseg_all, hs_d.ap().rearrange("(s p) n -> p s n", p=128))

            # ---------------- main loop ----------------
            for ch in range(NCH):
                wt4 = wt_tiles[ch]

                W1c = elem.tile([128, CJT, ISLICE], bf, tag="W1")
                nc.scalar.activation(W1c, wt4, AF.Relu)
                mskc = elem.tile([128, CJT, ISLICE], bf, tag="msk")
                nc.vector.tensor_scalar(mskc, W1c, 0.0, None, op0=OP.is_gt)

                # --- v and s_k for the CJT j-tiles of this chunk ---
                ps_v4 = pre4.tile([128, CJT, 256], f32, tag="v4")
                ps_sk4 = presk.tile([128, CJT, H], f32, tag="sk4")
                for jm in range(CJT):
                    jt = ch * CJT + jm
                    for a in range(2):
                        nc.tensor.matmul(ps_v4[:, jm, :],
                                         ht_sb[:, a, ts(jt, 128)],
                                         rhs_wv[:, a, :],
                                         start=(a == 0), stop=False)
                        nc.tensor.matmul(ps_sk4[:, jm, :],
                                         ht_sb[:, a, ts(jt, 128)],
                                         rhs_ak[:, a, :],
                                         start=(a == 0), stop=(a == 1))
                    nc.tensor.matmul(ps_v4[:, jm, :], ones_sb, bv_row,
                                     start=False, stop=True)

                g32 = gp.tile([128, CJT, H], f32, tag="g32")
                nc.scalar.activation(g32, ps_sk4, AF.Exp)

                rhs4 = rhsp.tile([128, CJT, 260], bf, tag="rhsbig")
                g32b = bass.AP(tensor=g32.tensor, offset=g32.offset,
                               ap=[g32.ap[0], g32.ap[1], g32.ap[2], [0, DH]])
                nc.vector.tensor_tensor(
                    out=rhs4[:, :, 0:256].rearrange(
                        "p j (h d) -> p j h d", h=H),
                    in0=ps_v4.rearrange("p j (h d) -> p j h d", h=H),
                    in1=g32b, op=OP.mult)
                nc.vector.tensor_copy(rhs4[:, :, 256:260], g32)

                st = (ch == 0)
                sp = (ch == NCH - 1)
                for jm in range(CJT):
                    for s in range(NSUB):
                        sl = ts(s, 128)
                        nc.tensor.matmul(psA[s][:, 0:260], W1c[:, jm, sl],
                                         rhs4[:, jm, :], start=st, stop=sp,
                                         skip_group_check=True)
                        nc.tensor.matmul(psA[s][:, 260:264], mskc[:, jm, sl],
                                         rhs4[:, jm, 256:260], start=st, stop=sp,
                                         skip_group_check=True)

            # ---------------- epilogue consts (end of sync queue) -------
            su2a = consts.tile([128, 896], bf, tag="su2a")
            nc.sync.dma_start(su2a, su2a_d.ap())
            su2b = consts.tile([128, 512], f32, tag="su2b")
            nc.sync.dma_start(su2b, su2b_d.ap())
            WoT_sb = su2a[:, 0:512].rearrange("p (a n) -> p a n", a=2)
            ident = su2a[:, 512:640]
            bo_row = su2a[0:1, 640:896]
            gam_sb = su2b[:, 0:256]
            bet_sb = su2b[:, 256:512]

            # ---------------- epilogue ----------------
            rdens = []
            for s in range(NSUB):
                dg = small.tile([128, H], f32, tag="dg", name=f"dg{s}")
                nc.vector.tensor_copy(dg, psA[s][:, 256:260])
                den = small.tile([128, H], f32, tag="den", name=f"den{s}")
                nc.vector.tensor_mul(den, c1b, dg)
                nc.vector.tensor_add(den, den, psA[s][:, 260:264])
                rden = small.tile([128, H], f32, tag="rden", name=f"rden{s}")
                nc.vector.reciprocal(rden, den)
                rdens.append(rden)

            msgs = []
            for s in range(NSUB):
                msg = outp.tile([128, D], bf, tag="msg", name=f"msg{s}")
                for h in range(H):
                    hsl = slice(h * DH, (h + 1) * DH)
                    nc.vector.tensor_scalar(msg[:, hsl], psA[s][:, hsl],
                                            rdens[s][:, h:h + 1], None,
                                            op0=OP.mult)
                msgs.append(msg)

            msgTs = []
            for s in range(NSUB):
                ps_t = pre4.tile([128, 2, 128], bf, tag="v4", name=f"pst{s}")
                for b in range(2):
                    nc.tensor.transpose(ps_t[:, b, :], msgs[s][:, ts(b, 128)],
                                        ident, )
                msgT = outp.tile([128, 2, 128], bf, tag="msgT", name=f"msgT{s}")
                nc.vector.tensor_copy(msgT, ps_t)
                msgTs.append(msgT)

            for s in range(NSUB):
                ps_o = pre4.tile([128, D], f32, tag="v4", name=f"pso{s}")
                nc.tensor.matmul(ps_o, msgTs[s][:, 0, :], WoT_sb[:, 0, :],
                                 start=True, stop=False)
                nc.tensor.matmul(ps_o, msgTs[s][:, 1, :], WoT_sb[:, 1, :],
                                 start=False, stop=False)
                nc.tensor.matmul(ps_o, ones_sb, bo_row, start=False, stop=True)

                x = outp.tile([128, D], f32, tag="x", name=f"x{s}")
                nc.vector.tensor_add(x, ps_o, hseg_all[:, s, :])

                stats = small.tile([128, 6], f32, tag="stats", name=f"st{s}")
                nc.vector.bn_stats(out=stats, in_=x)
                mv = small.tile([128, 2], f32, tag="mv", name=f"mv{s}")
                nc.vector.bn_aggr(out=mv, in_=stats)
                sd = small.tile([128, 1], f32, tag="sd", name=f"sd{s}")
                nc.scalar.activation(sd, mv[:, 1:2], AF.Sqrt, bias=eps_sb)
                rstd = small.tile([128, 1], f32, tag="rstd", name=f"rst{s}")
                nc.vector.reciprocal(rstd, sd)

                y = outp.tile([128, D], f32, tag="y", name=f"y{s}")
                nc.vector.tensor_scalar(y, x, mv[:, 0:1], rstd,
                                        op0=OP.subtract, op1=OP.mult)
                ot = outp.tile([128, D], f32, tag="ot", name=f"ot{s}")
                nc.vector.tensor_mul(ot, y, gam_sb)
                nc.vector.tensor_add(ot, ot, bet_sb)
                nc.sync.dma_start(out_d[ts(s, 128), :], ot)

    nc.compile()
    return nc


def _make_in_maps(h, w, Wk, Wv, bv, We_w, u, Wo, bo, gamma, beta, **_unused):
    import ml_dtypes
    f = np.float32
    b16 = ml_dtypes.bfloat16
    h = np.ascontiguousarray(h, dtype=f)
    wT = np.ascontiguousarray(np.asarray(w, dtype=f).T)
    Wk = np.asarray(Wk, dtype=f)
    u = np.asarray(u, dtype=f)
    We_w = np.asarray(We_w, dtype=f)

    # su1 (bf16): WvT 0:512 | Wk 512:1536 | u_k 1536:1540 | u_e 1540:1544
    #             | We_w 1544:1548 | bv row0 1548:1804
    su1 = np.zeros((128, 1804), f)
    WvT = np.asarray(Wv, dtype=f).T
    su1[:, 0:512] = WvT.reshape(2, 128, D).transpose(1, 0, 2).reshape(128, 512)
    for hh in range(H):
        su1[0:DH, 512 + hh * 256:512 + (hh + 1) * 256] = \
            Wk[hh * DH:(hh + 1) * DH, :]
        su1[0:DH, 1536 + hh] = u[hh, DH:2 * DH]
        su1[0:DE, 1540 + hh] = u[hh, 2 * DH:2 * DH + DE]
        su1[0:DE, 1544 + hh] = We_w[hh * DE:(hh + 1) * DE, 0]
    su1[0, 1548:1804] = np.asarray(bv, dtype=f)

    # su2a (bf16): WoT 0:512 | identity 512:640 | bo row0 640:896
    su2a = np.zeros((128, 896), f)
    WoT = np.asarray(Wo, dtype=f).T
    su2a[:, 0:512] = WoT.reshape(2, 128, D).transpose(1, 0, 2).reshape(128, 512)
    su2a[:, 512:640] = np.eye(128, dtype=f)
    su2a[0, 640:896] = np.asarray(bo, dtype=f)

    # su2b (f32): gamma/beta broadcast to 128 partitions
    su2b = np.zeros((128, 512), f)
    su2b[:, 0:256] = np.asarray(gamma, dtype=f)[None, :]
    su2b[:, 256:512] = np.asarray(beta, dtype=f)[None, :]

    common = {
        "ht": np.ascontiguousarray(h.T.astype(b16)),
        "su1": su1.astype(b16),
        "su2a": su2a.astype(b16),
        "su2b": su2b,
    }
    in_maps = []
    for c in range(NCORES):
        sl = slice(c * ISLICE, (c + 1) * ISLICE)
        m = dict(common)
        m["wt"] = np.ascontiguousarray(wT[:, sl])
        m["hs"] = np.ascontiguousarray(h[sl, :])
        in_maps.append(m)
    return in_maps


def kernel(**inputs):
    from concourse.bass_utils import run_bass_kernel_spmd

    if "nc" not in _cache:
        _cache["nc"] = _build_bass()
    nc = _cache["nc"]

    in_maps = _make_in_maps(**inputs)
    res = run_bass_kernel_spmd(nc, in_maps, core_ids=list(range(NCORES)))
    out = np.concatenate([r["out"] for r in res.results], axis=0)
    return np.ascontiguousarray(out, dtype=np.float32)
